# revision 19
# baseline (speedup 1.0000x reference)
"""AdaConv Trainium2 kernel (8 NeuronCores, group-sharded, v3).

Sharding: core c owns channel-GROUP c (64 channels) of ALL 8 samples
(instead of sample c).  The dk_w / pwk_w output-channel slices for
group c are exactly the data needed to build the effective 3x3 kernel
E = pk @ dw for group c of every sample, so there is NO collective:
each core predicts its group's kernels, combines them locally, and
convolves its 64-channel slab of all 8 samples.

Instance-norm is folded into the host-side pad+bf16-cast pass (mu and
sigma are per-sample scalars computed on host); the device epilogue is
a bias-only add (pb), split between the ACT and DVE engines so it can
never backpressure PSUM.

Per-core pipeline:
  sync  queue: wt blocks (16.8MB) interleaved with x tiles (17.3MB),
               then conv output (16.8MB).
  scalar queue: pkt / consts, per-block dw/pk transpose DMAs.
  Stage A GEMMs -> per-block transpose to [mc]-partition layout ->
  E^T = dw^T-matmul-pk^T (72 small matmuls) -> grouped 3x3 conv as
  4 concurrent 64x64 PE-array tiles, tap-outer accumulation in PSUM.
"""

import sys

if '/opt/trn_rl_repo' not in sys.path:
    sys.path.insert(0, '/opt/trn_rl_repo')

import numpy as np
import ml_dtypes

N_CORES = 8
C = 512
H = W = 128
PW = W + 2               # padded row length (130)
PHW = (H + 2) * PW       # padded channel image size (16900)
XAR = 68                 # rows in upper x sub-tile (chunks 0..20)
XBR = 67                 # rows in lower x sub-tile (chunks 21..42, base row 63)
XSPL = 21                # first chunk served by the lower sub-tile
XTW = (XAR + XBR) * PW   # per-pair xin width (17550)
RPC = 3                  # output rows per psum chunk
NCH = RPC * PW           # 390
NCHUNK = 43              # ceil(128/3): 42 chunks of 3 rows + 1 of 2
SCN = 3                  # chunks per super-chunk
OCS = 32768 // N_CORES   # dw/pk output-channel slice per core (4096)
KM = 2048                # dw predictor contraction (512ci * 2*2)
EPS = 1e-5

_CACHE = {}


def _build():
    import concourse.bacc as bacc
    import concourse.mybir as mybir
    import concourse.tile as tile

    f32 = mybir.dt.float32
    bf16 = mybir.dt.bfloat16
    ALU = mybir.AluOpType
    ACTF = mybir.ActivationFunctionType

    nc = bacc.Bacc("TRN2", target_bir_lowering=False, debug=False,
                   enable_asserts=True, num_devices=N_CORES)

    # ---- DRAM parameters (per-core shards prepared on host) ----
    xin = nc.dram_tensor("xin", [4, 128, XTW], bf16, kind="ExternalInput")
    wt = nc.dram_tensor("wt", [8, 2, 128, 4096], bf16, kind="ExternalInput")
    pkt = nc.dram_tensor("pkt", [8, 128, 2048], bf16, kind="ExternalInput")
    pbt = nc.dram_tensor("pbt", [128, 256], bf16, kind="ExternalInput")
    s_im = nc.dram_tensor("s_im", [16, 128, 72], bf16, kind="ExternalInput")
    sd_im = nc.dram_tensor("sd_im", [128, 32], bf16, kind="ExternalInput")
    dkb = nc.dram_tensor("dkb", [8, 512], bf16, kind="ExternalInput")
    pkb = nc.dram_tensor("pkb", [8, 512], bf16, kind="ExternalInput")
    pwbb = nc.dram_tensor("pwbb", [64, 1], f32, kind="ExternalInput")
    out = nc.dram_tensor("out", [4, 128, H * W], bf16, kind="ExternalOutput")
    dwd = nc.dram_tensor("dwd", [8, 72, 512], bf16)   # transpose bounce
    pkd = nc.dram_tensor("pkd", [8, 8, 512], bf16)

    with tile.TileContext(nc) as tc:
        with tc.tile_pool(name="const", bufs=1) as cpool, \
             tc.tile_pool(name="xblk", bufs=1) as xpool, \
             tc.tile_pool(name="epool", bufs=1) as epool, \
             tc.tile_pool(name="stg", bufs=1) as gpool:

            onesb = cpool.tile([1, 128], bf16)
            nc.vector.memset(onesb[:], 1.0)
            sd_b = cpool.tile([128, 32], bf16)
            nc.scalar.dma_start(sd_b[:], sd_im.ap())
            pbt_sb = cpool.tile([128, 256], bf16)
            nc.scalar.dma_start(pbt_sb[:], pbt.ap())
            pwbb_sb = cpool.tile([64, 1], f32)
            nc.scalar.dma_start(pwbb_sb[:], pwbb.ap())
            pbv = cpool.tile([128, 4], f32)     # epilogue bias per psum bank
            dwT = cpool.tile([64, 72 * 64], bf16)   # [mc, (n,t)*64+cl]
            pkT = cpool.tile([64, 512], bf16)       # [mc, n*64+oc]
            e_ts = [epool.tile([128, 9 * 128], bf16, name=f"e{qs}")
                    for qs in range(2)]

            # x tiles: pair p holds samples (2p, 2p+1), this core's 64ch.
            # Each pair is split into upper (rows 0..67) and lower (rows
            # 63..129) sub-tiles so the conv can start before the full
            # image has streamed in.
            xts = []
            for gp in range(4):
                xa = xpool.tile([128, XAR * PW], bf16, tag="xa", bufs=4,
                                name=f"xa{gp}")
                xb = xpool.tile([128, XBR * PW], bf16, tag="xb", bufs=4,
                                name=f"xb{gp}")
                xts.append((xa, xb))

            # ================= stage A (scoped pools) ======================
            with tc.tile_pool(name="sa", bufs=1) as apool, \
                 tc.tile_pool(name="wts", bufs=1) as wpool, \
                 tc.tile_pool(name="ps_a", bufs=1, space="PSUM") as ps_a:
                s_sb = apool.tile([128, 16 * 72], bf16)
                nc.sync.dma_start(
                    s_sb[:].rearrange("p (k c) -> p k c", k=16),
                    s_im.ap().rearrange("k p c -> p k c"))

                # ---- pb = pwb_w^T @ s_d + pwb_b  -> pbv [128, 4] ----------
                ps_pb = ps_a.tile([64, 8], f32, tag="psb", bufs=1)
                for kc in range(4):
                    nc.tensor.matmul(
                        ps_pb[:], pbt_sb[:, kc * 64:(kc + 1) * 64],
                        sd_b[:, kc * 8:(kc + 1) * 8],
                        start=(kc == 0), stop=(kc == 3))
                pb_f = apool.tile([64, 8], f32)
                nc.scalar.activation(pb_f[:], ps_pb[:], ACTF.Identity,
                                     bias=pwbb_sb[:], scale=1.0)
                # psA rows = samples (4qs+0 | 4qs+2); psB = (4qs+1 | 4qs+3)
                for qs in range(2):
                    nc.vector.tensor_copy(pbv[0:64, 2 * qs:2 * qs + 2],
                                          pb_f[:, 4 * qs:4 * qs + 2])
                    nc.vector.tensor_copy(pbv[64:128, 2 * qs:2 * qs + 2],
                                          pb_f[:, 4 * qs + 2:4 * qs + 4])

                for nch in range(8):
                    # bias tiles inline; bufs=4 so the dma only waits on a
                    # long-finished block
                    bia1 = wpool.tile([1, 512], bf16, tag="bia", bufs=4,
                                      name=f"dkb{nch}")
                    nc.scalar.dma_start(bia1[:], dkb.ap()[nch:nch + 1, :])
                    bia2 = wpool.tile([1, 512], bf16, tag="bi2", bufs=4,
                                      name=f"pkb{nch}")
                    nc.scalar.dma_start(bia2[:], pkb.ap()[nch:nch + 1, :])
                    # ---- dw slice block: [72=(n,t), 512=(mc_l,cl)] --------
                    ps_dw = ps_a.tile([72, 512], f32, tag="psa", bufs=3,
                                      name=f"psdw{nch}")
                    # quarter-size wt transfers with bufs=4 keep 2-3 DMAs
                    # in flight (a single transfer only reaches ~200GB/s)
                    for half in range(2):
                        for sub in range(2):
                            wq = wpool.tile([128, 2048], bf16, tag="wt",
                                            bufs=4,
                                            name=f"wt{nch}_{half}_{sub}")
                            nc.sync.dma_start(
                                wq[:],
                                wt.ap()[nch, half]
                                [:, sub * 2048:(sub + 1) * 2048])
                            for j2 in range(4):
                                kc = half * 8 + sub * 4 + j2
                                nc.tensor.matmul(
                                    ps_dw[:],
                                    s_sb[:, kc * 72:(kc + 1) * 72],
                                    wq[:, j2 * 512:(j2 + 1) * 512],
                                    start=(kc == 0), stop=False)
                    nc.tensor.matmul(ps_dw[:], onesb[0:1, 0:72],
                                     bia1[:], start=False, stop=True)
                    dw_blk = wpool.tile([72, 512], bf16, tag="dwb", bufs=2,
                                        name=f"dwb{nch}")
                    nc.vector.tensor_copy(dw_blk[:], ps_dw[:])
                    # transpose via DRAM bounce: flat DRAM APs have no
                    # partition-order constraint, so the read can iterate
                    # (m, r, c) and lowers to a few 2D descriptors.
                    nc.scalar.dma_start(dwd.ap()[nch], dw_blk[:])
                    nc.scalar.dma_start(
                        dwT[8 * nch:8 * nch + 8, :]
                        .rearrange("m (r c) -> m r c", c=64),
                        dwd.ap()[nch].rearrange("r (m c) -> m r c", m=8))

                    # ---- pk slice block: [8=n, 512=(mc_l,oc)] -------------
                    ps_pk = ps_a.tile([8, 512], f32, tag="psa", bufs=3,
                                      name=f"pspk{nch}")
                    pkt_sb = wpool.tile([128, 2048], bf16, tag="pkw",
                                        bufs=2, name=f"pkt{nch}")
                    nc.scalar.dma_start(pkt_sb[:], pkt.ap()[nch])
                    for kc in range(4):
                        nc.tensor.matmul(
                            ps_pk[:],
                            sd_b[:, kc * 8:(kc + 1) * 8],
                            pkt_sb[:, kc * 512:(kc + 1) * 512],
                            start=(kc == 0), stop=False)
                    nc.tensor.matmul(ps_pk[:], onesb[0:1, 0:8],
                                     bia2[:], start=False, stop=True)
                    pk_blk = wpool.tile([8, 512], bf16, tag="pkb", bufs=2,
                                        name=f"pkb{nch}")
                    nc.vector.tensor_copy(pk_blk[:], ps_pk[:])
                    nc.scalar.dma_start(pkd.ap()[nch], pk_blk[:])
                    nc.scalar.dma_start(
                        pkT[8 * nch:8 * nch + 8, :]
                        .rearrange("m (n o) -> m n o", o=64),
                        pkd.ap()[nch].rearrange("n (m o) -> m n o", m=8))

                # x after the weight stream; upper sub-tiles of the first
                # pairs first (those gate the conv start).  tile_wait_until
                # keeps the scheduler from front-running these 2.2MB
                # transfers into the middle of the wt stream.
                with tc.tile_wait_until(0.038):
                    for gp in range(4):
                        nc.sync.dma_start(xts[gp][0][:],
                                          xin.ap()[gp, :, 0:XAR * PW])
                with tc.tile_wait_until(0.052):
                    for gp in range(4):
                        nc.sync.dma_start(xts[gp][1][:],
                                          xin.ap()[gp, :, XAR * PW:XTW])

            # ============== E^T tiles: [cl, oc] per (sample, tap) ==========
            # e_ts[qs] layout: rows 0:64 = samples 4qs+0 (cols t*128+0:64)
            # and 4qs+2 (cols t*128+64:128); rows 64:128 = 4qs+1, 4qs+3.
            with tc.tile_pool(name="ps_e", bufs=1, space="PSUM") as ps_e:
                for qs in range(2):
                    for ch in range(2):
                        for tg in range(3):
                            psE = ps_e.tile([128, 192], f32, tag="pse",
                                            bufs=2, name=f"pse{qs}{ch}{tg}")
                            for tl in range(3):
                                t = 3 * tg + tl
                                ne = 4 * qs + 2 * ch
                                no = ne + 1
                                nc.tensor.matmul(
                                    psE[0:64, tl * 64:tl * 64 + 64],
                                    dwT[:, (ne * 9 + t) * 64:
                                        (ne * 9 + t) * 64 + 64],
                                    pkT[:, ne * 64:ne * 64 + 64],
                                    start=True, stop=True,
                                    tile_position=(0, 0))
                                nc.tensor.matmul(
                                    psE[64:128, tl * 64:tl * 64 + 64],
                                    dwT[:, (no * 9 + t) * 64:
                                        (no * 9 + t) * 64 + 64],
                                    pkT[:, no * 64:no * 64 + 64],
                                    start=True, stop=True,
                                    tile_position=(0, 64))
                            ev = e_ts[qs][:].rearrange(
                                "p (t x) -> p t x", x=128)
                            nc.vector.tensor_copy(
                                ev[:, 3 * tg:3 * tg + 3,
                                   ch * 64:ch * 64 + 64],
                                psE[:].rearrange("p (t x) -> p t x", x=64))

            # =================== conv (4-way PE tiling) ====================
            cvstack = tc.tile_pool(name="ps_cv", bufs=1, space="PSUM")
            pcv = cvstack.__enter__()
            for qs in range(2):
                et = e_ts[qs]
                nsc = (NCHUNK + SCN - 1) // SCN
                for sc in range(nsc):
                    gcs = list(range(sc * SCN, min((sc + 1) * SCN, NCHUNK)))
                    pas, pbs = [], []
                    for gc in gcs:
                        pas.append(pcv.tile([128, NCH], f32, tag="psA", bufs=4,
                                            name=f"cvA{qs}_{gc}"))
                        pbs.append(pcv.tile([128, NCH], f32, tag="psB", bufs=4,
                                            name=f"cvB{qs}_{gc}"))
                    for t in range(9):
                        i, j = t // 3, t % 3
                        st0, sp = (t == 0), (t == 8)
                        for k, gc in enumerate(gcs):
                            sub = 1 if gc >= XSPL else 0
                            XA = xts[2 * qs][sub]
                            XB = xts[2 * qs + 1][sub]
                            r0 = gc * RPC
                            nr = min(RPC, H - r0)
                            N = nr * PW - (2 if r0 + nr >= H else 0)
                            off = (r0 + i - (63 if sub else 0)) * PW + j
                            psA, psB = pas[k], pbs[k]
                            nc.tensor.matmul(
                                psA[0:64, 0:N], et[0:64, t * 128:t * 128 + 64],
                                XA[0:64, off:off + N], start=st0, stop=sp,
                                tile_position=(0, 0))
                            nc.tensor.matmul(
                                psB[0:64, 0:N],
                                et[64:128, t * 128:t * 128 + 64],
                                XA[64:128, off:off + N], start=st0, stop=sp,
                                tile_position=(64, 0))
                            nc.tensor.matmul(
                                psA[64:128, 0:N],
                                et[0:64, t * 128 + 64:t * 128 + 128],
                                XB[0:64, off:off + N], start=st0, stop=sp,
                                tile_position=(0, 64))
                            nc.tensor.matmul(
                                psB[64:128, 0:N],
                                et[64:128, t * 128 + 64:t * 128 + 128],
                                XB[64:128, off:off + N], start=st0, stop=sp,
                                tile_position=(64, 64))
                    # epilogue: strip halo cols, +bias, stage as bf16
                    # (split between ACT and DVE so neither backpressures)
                    stgA = gpool.tile([128, SCN * RPC * 128], bf16, tag="sgA",
                                      bufs=2, name=f"stA{qs}_{sc}")
                    stgB = gpool.tile([128, SCN * RPC * 128], bf16, tag="sgB",
                                      bufs=2, name=f"stB{qs}_{sc}")
                    cols = 0
                    for k, gc in enumerate(gcs):
                        nr = min(RPC, H - gc * RPC)
                        for ab, (ps, stg) in enumerate(
                                ((pas[k], stgA), (pbs[k], stgB))):
                            q = 2 * qs + ab
                            src = ps[:, 0:nr * PW].rearrange(
                                "p (r c) -> p r c", c=PW)[:, :, 0:128]
                            dst = stg[:, cols:cols + nr * 128].rearrange(
                                "p (r c) -> p r c", c=128)
                            if (k + ab) % 2 == 0:
                                nc.scalar.activation(dst, src, ACTF.Identity,
                                                     bias=pbv[:, q:q + 1],
                                                     scale=1.0)
                            else:
                                nc.vector.tensor_scalar(
                                    dst, src, pbv[:, q:q + 1], None,
                                    op0=ALU.add)
                        cols += nr * 128
                    o0 = sc * SCN * RPC * 128
                    nc.sync.dma_start(out.ap()[2 * qs, :, o0:o0 + cols],
                                      stgA[:, 0:cols])
                    nc.sync.dma_start(out.ap()[2 * qs + 1, :, o0:o0 + cols],
                                      stgB[:, 0:cols])
            cvstack.__exit__(None, None, None)

    nc.compile()
    return nc


def _host_prep(style_encoding, dk_w, dk_b, pwk_w, pwk_b, pwb_w, pwb_b):
    """Per-core weight shards (reshapes/transposes/casts only)."""
    f = np.float32
    bf = ml_dtypes.bfloat16
    st = np.asarray(style_encoding, f)                      # [8, 512, 4, 4]
    WTf = np.asarray(dk_w, f).reshape(32768, KM).T          # [2048, 32768]
    PKTf = np.asarray(pwk_w, f).reshape(32768, 512).T       # [512, 32768]
    pkb_f = np.asarray(pwk_b, f)
    PBT = np.ascontiguousarray(np.asarray(pwb_w, f).reshape(512, 512).T)
    pwb_bf = np.asarray(pwb_b, f)
    dkb_f = np.asarray(dk_b, f)

    # style-tap matrix for the dw GEMM: rows k = ci*4 + khw, cols = n*9 + t
    S = np.empty((KM, 72), f)
    for kh in range(2):
        for kw in range(2):
            blk = st[:, :, kh:kh + 3, kw:kw + 3].reshape(8, 512, 9)
            S[kh * 2 + kw::4, :] = blk.transpose(1, 0, 2).reshape(512, 72)
    S = np.ascontiguousarray(S.reshape(16, 128, 72)).astype(bf)

    # s_d (global mean of the 4x4 style map): [128, kc*8 + n]
    sdvec = st.mean(axis=(2, 3))                            # [8, 512]
    sd_g = np.ascontiguousarray(
        sdvec.T.reshape(4, 128, 8).transpose(1, 0, 2)).reshape(128, 32)
    sd_g = sd_g.astype(bf)

    shards = []
    for g in range(N_CORES):
        sl = slice(g * OCS, (g + 1) * OCS)
        # dw weights: [nch, half, 128, (k8, 512)] — cols (mc_l, cl)
        wtg = np.ascontiguousarray(
            WTf[:, sl].reshape(2, 8, 128, 8, 512).transpose(3, 0, 2, 1, 4)
        ).reshape(8, 2, 128, 4096).astype(bf)
        # pk weights: device cols (mc_l, oc) per block (mc = 8*nch + mc_l)
        PKc = PKTf[:, sl].reshape(512, 64, 64)              # [sd, oc, mc]
        PKp = PKc.transpose(0, 2, 1).reshape(512, 8, 512)   # [sd, b, (m,oc)]
        pktg = np.ascontiguousarray(
            PKp.reshape(4, 128, 8, 512).transpose(2, 1, 0, 3)
        ).reshape(8, 128, 2048).astype(bf)
        pkb_g = np.ascontiguousarray(
            pkb_f[sl].reshape(64, 64).T).reshape(8, 512).astype(bf)
        # pb predictor slice: [128, kc*64 + oc]
        pbt_g = np.ascontiguousarray(
            PBT[:, g * 64:(g + 1) * 64].reshape(4, 128, 64)
            .transpose(1, 0, 2)).reshape(128, 256).astype(bf)
        pwbb_g = np.ascontiguousarray(
            pwb_bf[g * 64:(g + 1) * 64].reshape(64, 1))
        shards.append(dict(
            wt=wtg, pkt=pktg, pbt=pbt_g, s_im=S, sd_im=sd_g,
            dkb=np.ascontiguousarray(dkb_f[sl]).reshape(8, 512).astype(bf),
            pkb=pkb_g, pwbb=pwbb_g,
        ))
    return shards


def _prep_x(predicted, norm):
    """Normalize+pad+cast on host -> per-core [4, 128, XTW] bf16 tiles
    (rows 0..67 then rows 63..129 of the padded image, per sample pair)."""
    f = np.float32
    bf = ml_dtypes.bfloat16
    x = np.asarray(predicted, f).reshape(N_CORES, C, H, W)
    if norm:
        mu = x.mean(axis=(1, 2, 3), keepdims=True)
        sd = np.sqrt(x.var(axis=(1, 2, 3), keepdims=True) + EPS)
        x = (x - mu) / sd
    xp = np.pad(x, ((0, 0), (0, 0), (1, 1), (1, 1)), mode='reflect')
    xp = xp.astype(bf)                                       # [8,512,130,130]
    xins = []
    for g in range(N_CORES):
        # pair p holds samples (2p, 2p+1), channels [64g, 64g+64)
        xg = xp[:, 64 * g:64 * g + 64].reshape(4, 128, H + 2, PW)
        xo = np.empty((4, 128, XTW), bf)
        xo[:, :, 0:XAR * PW] = xg[:, :, 0:XAR].reshape(4, 128, XAR * PW)
        xo[:, :, XAR * PW:] = xg[:, :, 63:63 + XBR].reshape(4, 128, XBR * PW)
        xins.append(xo)
    return xins


def kernel(style_encoding, predicted, dk_w, dk_b, pwk_w, pwk_b, pwb_w, pwb_b,
           norm=True, **_ignored):
    from concourse import bass_utils

    norm = bool(norm)
    if "nc" not in _CACHE:
        _CACHE["nc"] = _build()
    nc = _CACHE["nc"]

    shards = _host_prep(style_encoding, dk_w, dk_b, pwk_w, pwk_b,
                        pwb_w, pwb_b)
    xins = _prep_x(predicted, norm)
    in_maps = []
    for g in range(N_CORES):
        m = dict(shards[g])
        m["xin"] = xins[g]
        in_maps.append(m)

    res = bass_utils.run_bass_kernel_spmd(nc, in_maps,
                                          core_ids=list(range(N_CORES)))
    return _gather(res)


def _gather(res):
    out = np.empty((N_CORES, C, H * W), np.float32)
    for g in range(N_CORES):
        ob = np.asarray(res.results[g]["out"]).astype(np.float32)
        for qs in range(2):
            out[4 * qs + 0, 64 * g:64 * g + 64] = ob[2 * qs, 0:64]
            out[4 * qs + 2, 64 * g:64 * g + 64] = ob[2 * qs, 64:128]
            out[4 * qs + 1, 64 * g:64 * g + 64] = ob[2 * qs + 1, 0:64]
            out[4 * qs + 3, 64 * g:64 * g + 64] = ob[2 * qs + 1, 64:128]
    return out.reshape(N_CORES, C, H, W)


# revision 21
# speedup vs baseline: 1.0341x; 1.0341x over previous
"""AdaConv Trainium2 kernel (8 NeuronCores, group-sharded, v3).

Sharding: core c owns channel-GROUP c (64 channels) of ALL 8 samples
(instead of sample c).  The dk_w / pwk_w output-channel slices for
group c are exactly the data needed to build the effective 3x3 kernel
E = pk @ dw for group c of every sample, so there is NO collective:
each core predicts its group's kernels, combines them locally, and
convolves its 64-channel slab of all 8 samples.

Instance-norm is folded into the host-side pad+bf16-cast pass (mu and
sigma are per-sample scalars computed on host); the device epilogue is
a bias-only add (pb), split between the ACT and DVE engines so it can
never backpressure PSUM.

Per-core pipeline:
  sync  queue: wt blocks (16.8MB) interleaved with x tiles (17.3MB),
               then conv output (16.8MB).
  scalar queue: pkt / consts, per-block dw/pk transpose DMAs.
  Stage A GEMMs -> per-block transpose to [mc]-partition layout ->
  E^T = dw^T-matmul-pk^T (72 small matmuls) -> grouped 3x3 conv as
  4 concurrent 64x64 PE-array tiles, tap-outer accumulation in PSUM.
"""

import sys

if '/opt/trn_rl_repo' not in sys.path:
    sys.path.insert(0, '/opt/trn_rl_repo')

import numpy as np
import ml_dtypes

N_CORES = 8
C = 512
H = W = 128
PW = W + 2               # padded row length (130)
PHW = (H + 2) * PW       # padded channel image size (16900)
XAR = 68                 # rows in upper x sub-tile (chunks 0..20)
XBR = 67                 # rows in lower x sub-tile (chunks 21..42, base row 63)
XSPL = 21                # first chunk served by the lower sub-tile
XTW = (XAR + XBR) * PW   # per-pair xin width (17550)
RPC = 3                  # output rows per psum chunk
NCH = RPC * PW           # 390
NCHUNK = 43              # ceil(128/3): 42 chunks of 3 rows + 1 of 2
SCN = 3                  # chunks per super-chunk
OCS = 32768 // N_CORES   # dw/pk output-channel slice per core (4096)
KM = 2048                # dw predictor contraction (512ci * 2*2)
EPS = 1e-5

_CACHE = {}


def _build():
    import concourse.bacc as bacc
    import concourse.mybir as mybir
    import concourse.tile as tile

    f32 = mybir.dt.float32
    bf16 = mybir.dt.bfloat16
    ALU = mybir.AluOpType
    ACTF = mybir.ActivationFunctionType

    nc = bacc.Bacc("TRN2", target_bir_lowering=False, debug=False,
                   enable_asserts=True, num_devices=N_CORES)

    # ---- DRAM parameters (per-core shards prepared on host) ----
    xin = nc.dram_tensor("xin", [4, 128, XTW], bf16, kind="ExternalInput")
    wt = nc.dram_tensor("wt", [8, 2, 128, 4096], bf16, kind="ExternalInput")
    pkt = nc.dram_tensor("pkt", [8, 128, 2048], bf16, kind="ExternalInput")
    pbt = nc.dram_tensor("pbt", [128, 256], bf16, kind="ExternalInput")
    s_im = nc.dram_tensor("s_im", [16, 128, 72], bf16, kind="ExternalInput")
    sd_im = nc.dram_tensor("sd_im", [128, 32], bf16, kind="ExternalInput")
    dkb = nc.dram_tensor("dkb", [8, 512], bf16, kind="ExternalInput")
    pkb = nc.dram_tensor("pkb", [8, 512], bf16, kind="ExternalInput")
    pwbb = nc.dram_tensor("pwbb", [64, 1], f32, kind="ExternalInput")
    out = nc.dram_tensor("out", [4, 128, H * W], bf16, kind="ExternalOutput")
    dwd = nc.dram_tensor("dwd", [8, 72, 512], bf16)   # transpose bounce
    pkd = nc.dram_tensor("pkd", [8, 8, 512], bf16)

    with tile.TileContext(nc) as tc:
        with tc.tile_pool(name="const", bufs=1) as cpool, \
             tc.tile_pool(name="xblk", bufs=1) as xpool, \
             tc.tile_pool(name="epool", bufs=1) as epool, \
             tc.tile_pool(name="stg", bufs=1) as gpool:

            onesb = cpool.tile([1, 128], bf16)
            nc.vector.memset(onesb[:], 1.0)
            sd_b = cpool.tile([128, 32], bf16)
            nc.scalar.dma_start(sd_b[:], sd_im.ap())
            pbt_sb = cpool.tile([128, 256], bf16)
            nc.scalar.dma_start(pbt_sb[:], pbt.ap())
            pwbb_sb = cpool.tile([64, 1], f32)
            nc.scalar.dma_start(pwbb_sb[:], pwbb.ap())
            pbv = cpool.tile([128, 4], f32)     # epilogue bias per psum bank
            dwT = cpool.tile([64, 72 * 64], bf16)   # [mc, (n,t)*64+cl]
            pkT = cpool.tile([64, 512], bf16)       # [mc, n*64+oc]
            e_ts = [epool.tile([128, 9 * 128], bf16, name=f"e{qs}")
                    for qs in range(2)]

            # x tiles: pair p holds samples (2p, 2p+1), this core's 64ch.
            # Each pair is split into upper (rows 0..67) and lower (rows
            # 63..129) sub-tiles so the conv can start before the full
            # image has streamed in.
            xts = []
            for gp in range(4):
                xa = xpool.tile([128, XAR * PW], bf16, tag="xa", bufs=4,
                                name=f"xa{gp}")
                xb = xpool.tile([128, XBR * PW], bf16, tag="xb", bufs=4,
                                name=f"xb{gp}")
                xts.append((xa, xb))

            # ================= stage A (scoped pools) ======================
            with tc.tile_pool(name="sa", bufs=1) as apool, \
                 tc.tile_pool(name="wts", bufs=1) as wpool, \
                 tc.tile_pool(name="ps_a", bufs=1, space="PSUM") as ps_a:
                s_sb = apool.tile([128, 16 * 72], bf16)
                nc.sync.dma_start(
                    s_sb[:].rearrange("p (k c) -> p k c", k=16),
                    s_im.ap().rearrange("k p c -> p k c"))

                # ---- pb = pwb_w^T @ s_d + pwb_b  -> pbv [128, 4] ----------
                ps_pb = ps_a.tile([64, 8], f32, tag="psb", bufs=1)
                for kc in range(4):
                    nc.tensor.matmul(
                        ps_pb[:], pbt_sb[:, kc * 64:(kc + 1) * 64],
                        sd_b[:, kc * 8:(kc + 1) * 8],
                        start=(kc == 0), stop=(kc == 3))
                pb_f = apool.tile([64, 8], f32)
                nc.scalar.activation(pb_f[:], ps_pb[:], ACTF.Identity,
                                     bias=pwbb_sb[:], scale=1.0)
                # psA rows = samples (4qs+0 | 4qs+2); psB = (4qs+1 | 4qs+3)
                for qs in range(2):
                    nc.vector.tensor_copy(pbv[0:64, 2 * qs:2 * qs + 2],
                                          pb_f[:, 4 * qs:4 * qs + 2])
                    nc.vector.tensor_copy(pbv[64:128, 2 * qs:2 * qs + 2],
                                          pb_f[:, 4 * qs + 2:4 * qs + 4])

                for nch in range(8):
                    # bias tiles inline; bufs=4 so the dma only waits on a
                    # long-finished block
                    bia1 = wpool.tile([1, 512], bf16, tag="bia", bufs=3,
                                      name=f"dkb{nch}")
                    nc.scalar.dma_start(bia1[:], dkb.ap()[nch:nch + 1, :])
                    bia2 = wpool.tile([1, 512], bf16, tag="bi2", bufs=3,
                                      name=f"pkb{nch}")
                    nc.scalar.dma_start(bia2[:], pkb.ap()[nch:nch + 1, :])
                    # ---- dw slice block: [72=(n,t), 512=(mc_l,cl)] --------
                    ps_dw = ps_a.tile([72, 512], f32, tag="psa", bufs=3,
                                      name=f"psdw{nch}")
                    # half-block wt transfers (8KB/partition runs) with
                    # bufs=3 so ~2 DMAs stay in flight — bigger runs give
                    # much better per-descriptor DMA efficiency
                    for half in range(2):
                        wq = wpool.tile([128, 4096], bf16, tag="wt",
                                        bufs=3, name=f"wt{nch}_{half}")
                        nc.sync.dma_start(wq[:], wt.ap()[nch, half])
                        for k8 in range(8):
                            kc = half * 8 + k8
                            nc.tensor.matmul(
                                ps_dw[:],
                                s_sb[:, kc * 72:(kc + 1) * 72],
                                wq[:, k8 * 512:(k8 + 1) * 512],
                                start=(kc == 0), stop=False)
                    nc.tensor.matmul(ps_dw[:], onesb[0:1, 0:72],
                                     bia1[:], start=False, stop=True)
                    dw_blk = wpool.tile([72, 512], bf16, tag="dwb", bufs=2,
                                        name=f"dwb{nch}")
                    nc.vector.tensor_copy(dw_blk[:], ps_dw[:])
                    # transpose via DRAM bounce: flat DRAM APs have no
                    # partition-order constraint, so the read can iterate
                    # (m, r, c) and lowers to a few 2D descriptors.
                    nc.scalar.dma_start(dwd.ap()[nch], dw_blk[:])
                    nc.scalar.dma_start(
                        dwT[8 * nch:8 * nch + 8, :]
                        .rearrange("m (r c) -> m r c", c=64),
                        dwd.ap()[nch].rearrange("r (m c) -> m r c", m=8))

                    # ---- pk slice block: [8=n, 512=(mc_l,oc)] -------------
                    ps_pk = ps_a.tile([8, 512], f32, tag="psa", bufs=3,
                                      name=f"pspk{nch}")
                    pkt_sb = wpool.tile([128, 2048], bf16, tag="pkw",
                                        bufs=2, name=f"pkt{nch}")
                    nc.scalar.dma_start(pkt_sb[:], pkt.ap()[nch])
                    for kc in range(4):
                        nc.tensor.matmul(
                            ps_pk[:],
                            sd_b[:, kc * 8:(kc + 1) * 8],
                            pkt_sb[:, kc * 512:(kc + 1) * 512],
                            start=(kc == 0), stop=False)
                    nc.tensor.matmul(ps_pk[:], onesb[0:1, 0:8],
                                     bia2[:], start=False, stop=True)
                    pk_blk = wpool.tile([8, 512], bf16, tag="pkb", bufs=2,
                                        name=f"pkb{nch}")
                    nc.vector.tensor_copy(pk_blk[:], ps_pk[:])
                    nc.scalar.dma_start(pkd.ap()[nch], pk_blk[:])
                    nc.scalar.dma_start(
                        pkT[8 * nch:8 * nch + 8, :]
                        .rearrange("m (n o) -> m n o", o=64),
                        pkd.ap()[nch].rearrange("n (m o) -> m n o", m=8))

                # x after the weight stream; upper sub-tiles of the first
                # pairs first (those gate the conv start).  tile_wait_until
                # keeps the scheduler from front-running these 2.2MB
                # transfers into the middle of the wt stream.
                with tc.tile_wait_until(0.038):
                    for gp in range(4):
                        nc.sync.dma_start(xts[gp][0][:],
                                          xin.ap()[gp, :, 0:XAR * PW])
                with tc.tile_wait_until(0.052):
                    for gp in range(4):
                        nc.sync.dma_start(xts[gp][1][:],
                                          xin.ap()[gp, :, XAR * PW:XTW])

            # ============== E^T tiles: [cl, oc] per (sample, tap) ==========
            # e_ts[qs] layout: rows 0:64 = samples 4qs+0 (cols t*128+0:64)
            # and 4qs+2 (cols t*128+64:128); rows 64:128 = 4qs+1, 4qs+3.
            with tc.tile_pool(name="ps_e", bufs=1, space="PSUM") as ps_e:
                for qs in range(2):
                    for ch in range(2):
                        for tg in range(3):
                            psE = ps_e.tile([128, 192], f32, tag="pse",
                                            bufs=2, name=f"pse{qs}{ch}{tg}")
                            for tl in range(3):
                                t = 3 * tg + tl
                                ne = 4 * qs + 2 * ch
                                no = ne + 1
                                nc.tensor.matmul(
                                    psE[0:64, tl * 64:tl * 64 + 64],
                                    dwT[:, (ne * 9 + t) * 64:
                                        (ne * 9 + t) * 64 + 64],
                                    pkT[:, ne * 64:ne * 64 + 64],
                                    start=True, stop=True,
                                    tile_position=(0, 0))
                                nc.tensor.matmul(
                                    psE[64:128, tl * 64:tl * 64 + 64],
                                    dwT[:, (no * 9 + t) * 64:
                                        (no * 9 + t) * 64 + 64],
                                    pkT[:, no * 64:no * 64 + 64],
                                    start=True, stop=True,
                                    tile_position=(0, 64))
                            ev = e_ts[qs][:].rearrange(
                                "p (t x) -> p t x", x=128)
                            nc.vector.tensor_copy(
                                ev[:, 3 * tg:3 * tg + 3,
                                   ch * 64:ch * 64 + 64],
                                psE[:].rearrange("p (t x) -> p t x", x=64))

            # =================== conv (4-way PE tiling) ====================
            cvstack = tc.tile_pool(name="ps_cv", bufs=1, space="PSUM")
            pcv = cvstack.__enter__()
            for qs in range(2):
                et = e_ts[qs]
                nsc = (NCHUNK + SCN - 1) // SCN
                for sc in range(nsc):
                    gcs = list(range(sc * SCN, min((sc + 1) * SCN, NCHUNK)))
                    pas, pbs = [], []
                    for gc in gcs:
                        pas.append(pcv.tile([128, NCH], f32, tag="psA", bufs=4,
                                            name=f"cvA{qs}_{gc}"))
                        pbs.append(pcv.tile([128, NCH], f32, tag="psB", bufs=4,
                                            name=f"cvB{qs}_{gc}"))
                    for t in range(9):
                        i, j = t // 3, t % 3
                        st0, sp = (t == 0), (t == 8)
                        for k, gc in enumerate(gcs):
                            sub = 1 if gc >= XSPL else 0
                            XA = xts[2 * qs][sub]
                            XB = xts[2 * qs + 1][sub]
                            r0 = gc * RPC
                            nr = min(RPC, H - r0)
                            N = nr * PW - (2 if r0 + nr >= H else 0)
                            off = (r0 + i - (63 if sub else 0)) * PW + j
                            psA, psB = pas[k], pbs[k]
                            nc.tensor.matmul(
                                psA[0:64, 0:N], et[0:64, t * 128:t * 128 + 64],
                                XA[0:64, off:off + N], start=st0, stop=sp,
                                tile_position=(0, 0))
                            nc.tensor.matmul(
                                psB[0:64, 0:N],
                                et[64:128, t * 128:t * 128 + 64],
                                XA[64:128, off:off + N], start=st0, stop=sp,
                                tile_position=(64, 0))
                            nc.tensor.matmul(
                                psA[64:128, 0:N],
                                et[0:64, t * 128 + 64:t * 128 + 128],
                                XB[0:64, off:off + N], start=st0, stop=sp,
                                tile_position=(0, 64))
                            nc.tensor.matmul(
                                psB[64:128, 0:N],
                                et[64:128, t * 128 + 64:t * 128 + 128],
                                XB[64:128, off:off + N], start=st0, stop=sp,
                                tile_position=(64, 64))
                    # epilogue: strip halo cols, +bias, stage as bf16
                    # (split between ACT and DVE so neither backpressures)
                    stgA = gpool.tile([128, SCN * RPC * 128], bf16, tag="sgA",
                                      bufs=2, name=f"stA{qs}_{sc}")
                    stgB = gpool.tile([128, SCN * RPC * 128], bf16, tag="sgB",
                                      bufs=2, name=f"stB{qs}_{sc}")
                    cols = 0
                    for k, gc in enumerate(gcs):
                        nr = min(RPC, H - gc * RPC)
                        for ab, (ps, stg) in enumerate(
                                ((pas[k], stgA), (pbs[k], stgB))):
                            q = 2 * qs + ab
                            src = ps[:, 0:nr * PW].rearrange(
                                "p (r c) -> p r c", c=PW)[:, :, 0:128]
                            dst = stg[:, cols:cols + nr * 128].rearrange(
                                "p (r c) -> p r c", c=128)
                            if (k + ab) % 2 == 0:
                                nc.scalar.activation(dst, src, ACTF.Identity,
                                                     bias=pbv[:, q:q + 1],
                                                     scale=1.0)
                            else:
                                nc.vector.tensor_scalar(
                                    dst, src, pbv[:, q:q + 1], None,
                                    op0=ALU.add)
                        cols += nr * 128
                    o0 = sc * SCN * RPC * 128
                    nc.sync.dma_start(out.ap()[2 * qs, :, o0:o0 + cols],
                                      stgA[:, 0:cols])
                    nc.sync.dma_start(out.ap()[2 * qs + 1, :, o0:o0 + cols],
                                      stgB[:, 0:cols])
            cvstack.__exit__(None, None, None)

    nc.compile()
    return nc


def _host_prep(style_encoding, dk_w, dk_b, pwk_w, pwk_b, pwb_w, pwb_b):
    """Per-core weight shards (reshapes/transposes/casts only)."""
    f = np.float32
    bf = ml_dtypes.bfloat16
    st = np.asarray(style_encoding, f)                      # [8, 512, 4, 4]
    WTf = np.asarray(dk_w, f).reshape(32768, KM).T          # [2048, 32768]
    PKTf = np.asarray(pwk_w, f).reshape(32768, 512).T       # [512, 32768]
    pkb_f = np.asarray(pwk_b, f)
    PBT = np.ascontiguousarray(np.asarray(pwb_w, f).reshape(512, 512).T)
    pwb_bf = np.asarray(pwb_b, f)
    dkb_f = np.asarray(dk_b, f)

    # style-tap matrix for the dw GEMM: rows k = ci*4 + khw, cols = n*9 + t
    S = np.empty((KM, 72), f)
    for kh in range(2):
        for kw in range(2):
            blk = st[:, :, kh:kh + 3, kw:kw + 3].reshape(8, 512, 9)
            S[kh * 2 + kw::4, :] = blk.transpose(1, 0, 2).reshape(512, 72)
    S = np.ascontiguousarray(S.reshape(16, 128, 72)).astype(bf)

    # s_d (global mean of the 4x4 style map): [128, kc*8 + n]
    sdvec = st.mean(axis=(2, 3))                            # [8, 512]
    sd_g = np.ascontiguousarray(
        sdvec.T.reshape(4, 128, 8).transpose(1, 0, 2)).reshape(128, 32)
    sd_g = sd_g.astype(bf)

    shards = []
    for g in range(N_CORES):
        sl = slice(g * OCS, (g + 1) * OCS)
        # dw weights: [nch, half, 128, (k8, 512)] — cols (mc_l, cl)
        wtg = np.ascontiguousarray(
            WTf[:, sl].reshape(2, 8, 128, 8, 512).transpose(3, 0, 2, 1, 4)
        ).reshape(8, 2, 128, 4096).astype(bf)
        # pk weights: device cols (mc_l, oc) per block (mc = 8*nch + mc_l)
        PKc = PKTf[:, sl].reshape(512, 64, 64)              # [sd, oc, mc]
        PKp = PKc.transpose(0, 2, 1).reshape(512, 8, 512)   # [sd, b, (m,oc)]
        pktg = np.ascontiguousarray(
            PKp.reshape(4, 128, 8, 512).transpose(2, 1, 0, 3)
        ).reshape(8, 128, 2048).astype(bf)
        pkb_g = np.ascontiguousarray(
            pkb_f[sl].reshape(64, 64).T).reshape(8, 512).astype(bf)
        # pb predictor slice: [128, kc*64 + oc]
        pbt_g = np.ascontiguousarray(
            PBT[:, g * 64:(g + 1) * 64].reshape(4, 128, 64)
            .transpose(1, 0, 2)).reshape(128, 256).astype(bf)
        pwbb_g = np.ascontiguousarray(
            pwb_bf[g * 64:(g + 1) * 64].reshape(64, 1))
        shards.append(dict(
            wt=wtg, pkt=pktg, pbt=pbt_g, s_im=S, sd_im=sd_g,
            dkb=np.ascontiguousarray(dkb_f[sl]).reshape(8, 512).astype(bf),
            pkb=pkb_g, pwbb=pwbb_g,
        ))
    return shards


def _prep_x(predicted, norm):
    """Normalize+pad+cast on host -> per-core [4, 128, XTW] bf16 tiles
    (rows 0..67 then rows 63..129 of the padded image, per sample pair)."""
    f = np.float32
    bf = ml_dtypes.bfloat16
    x = np.asarray(predicted, f).reshape(N_CORES, C, H, W)
    if norm:
        mu = x.mean(axis=(1, 2, 3), keepdims=True)
        sd = np.sqrt(x.var(axis=(1, 2, 3), keepdims=True) + EPS)
        x = (x - mu) / sd
    xp = np.pad(x, ((0, 0), (0, 0), (1, 1), (1, 1)), mode='reflect')
    xp = xp.astype(bf)                                       # [8,512,130,130]
    xins = []
    for g in range(N_CORES):
        # pair p holds samples (2p, 2p+1), channels [64g, 64g+64)
        xg = xp[:, 64 * g:64 * g + 64].reshape(4, 128, H + 2, PW)
        xo = np.empty((4, 128, XTW), bf)
        xo[:, :, 0:XAR * PW] = xg[:, :, 0:XAR].reshape(4, 128, XAR * PW)
        xo[:, :, XAR * PW:] = xg[:, :, 63:63 + XBR].reshape(4, 128, XBR * PW)
        xins.append(xo)
    return xins


def kernel(style_encoding, predicted, dk_w, dk_b, pwk_w, pwk_b, pwb_w, pwb_b,
           norm=True, **_ignored):
    from concourse import bass_utils

    norm = bool(norm)
    if "nc" not in _CACHE:
        _CACHE["nc"] = _build()
    nc = _CACHE["nc"]

    shards = _host_prep(style_encoding, dk_w, dk_b, pwk_w, pwk_b,
                        pwb_w, pwb_b)
    xins = _prep_x(predicted, norm)
    in_maps = []
    for g in range(N_CORES):
        m = dict(shards[g])
        m["xin"] = xins[g]
        in_maps.append(m)

    res = bass_utils.run_bass_kernel_spmd(nc, in_maps,
                                          core_ids=list(range(N_CORES)))
    return _gather(res)


def _gather(res):
    out = np.empty((N_CORES, C, H * W), np.float32)
    for g in range(N_CORES):
        ob = np.asarray(res.results[g]["out"]).astype(np.float32)
        for qs in range(2):
            out[4 * qs + 0, 64 * g:64 * g + 64] = ob[2 * qs, 0:64]
            out[4 * qs + 2, 64 * g:64 * g + 64] = ob[2 * qs, 64:128]
            out[4 * qs + 1, 64 * g:64 * g + 64] = ob[2 * qs + 1, 0:64]
            out[4 * qs + 3, 64 * g:64 * g + 64] = ob[2 * qs + 1, 64:128]
    return out.reshape(N_CORES, C, H, W)


# revision 31
# speedup vs baseline: 1.0372x; 1.0030x over previous
"""AdaConv Trainium2 kernel (8 NeuronCores, group-sharded, v3).

Sharding: core c owns channel-GROUP c (64 channels) of ALL 8 samples
(instead of sample c).  The dk_w / pwk_w output-channel slices for
group c are exactly the data needed to build the effective 3x3 kernel
E = pk @ dw for group c of every sample, so there is NO collective:
each core predicts its group's kernels, combines them locally, and
convolves its 64-channel slab of all 8 samples.

Instance-norm is folded into the host-side pad+bf16-cast pass (mu and
sigma are per-sample scalars computed on host); the device epilogue is
a bias-only add (pb), split between the ACT and DVE engines so it can
never backpressure PSUM.

Per-core pipeline:
  sync  queue: wt blocks (16.8MB) interleaved with x tiles (17.3MB),
               then conv output (16.8MB).
  scalar queue: pkt / consts, per-block dw/pk transpose DMAs.
  Stage A GEMMs -> per-block transpose to [mc]-partition layout ->
  E^T = dw^T-matmul-pk^T (72 small matmuls) -> grouped 3x3 conv as
  4 concurrent 64x64 PE-array tiles, tap-outer accumulation in PSUM.
"""

import sys

if '/opt/trn_rl_repo' not in sys.path:
    sys.path.insert(0, '/opt/trn_rl_repo')

import numpy as np
import ml_dtypes

N_CORES = 8
C = 512
H = W = 128
PW = W + 2               # padded row length (130)
PHW = (H + 2) * PW       # padded channel image size (16900)
# x row-bands: band b serves chunks 11b..11b+10 and holds padded rows
# [33b, 33b+35) (+2 halo elems; band 3: 31 rows) — 2-row overlaps let
# the conv start per band
XBASE = [0, 33, 66, 99]
XW = [35 * PW + 2, 35 * PW + 2, 35 * PW + 2, 31 * PW]
XTW = sum(XW)            # per-pair xin width (17686)
RPC = 3                  # output rows per psum chunk
NCH = RPC * PW           # 390
NCHUNK = 43              # ceil(128/3): 42 chunks of 3 rows + 1 of 2
SCN = 3                  # chunks per super-chunk
OCS = 32768 // N_CORES   # dw/pk output-channel slice per core (4096)
KM = 2048                # dw predictor contraction (512ci * 2*2)
EPS = 1e-5

_CACHE = {}


def _build():
    import concourse.bacc as bacc
    import concourse.mybir as mybir
    import concourse.tile as tile

    f32 = mybir.dt.float32
    bf16 = mybir.dt.bfloat16
    ALU = mybir.AluOpType
    ACTF = mybir.ActivationFunctionType

    nc = bacc.Bacc("TRN2", target_bir_lowering=False, debug=False,
                   enable_asserts=True, num_devices=N_CORES)

    # ---- DRAM parameters (per-core shards prepared on host) ----
    xin = nc.dram_tensor("xin", [4, 128, XTW], bf16, kind="ExternalInput")
    wt = nc.dram_tensor("wt", [8, 2, 128, 4096], bf16, kind="ExternalInput")
    pkt = nc.dram_tensor("pkt", [8, 128, 2048], bf16, kind="ExternalInput")
    pbt = nc.dram_tensor("pbt", [128, 256], bf16, kind="ExternalInput")
    s_im = nc.dram_tensor("s_im", [16, 128, 72], bf16, kind="ExternalInput")
    sd_im = nc.dram_tensor("sd_im", [128, 32], bf16, kind="ExternalInput")
    dkb = nc.dram_tensor("dkb", [8, 512], bf16, kind="ExternalInput")
    pkb = nc.dram_tensor("pkb", [8, 512], bf16, kind="ExternalInput")
    pwbb = nc.dram_tensor("pwbb", [64, 1], f32, kind="ExternalInput")
    out = nc.dram_tensor("out", [4, 128, H * W], bf16, kind="ExternalOutput")
    dwd = nc.dram_tensor("dwd", [8, 72, 512], bf16)   # transpose bounce
    pkd = nc.dram_tensor("pkd", [8, 8, 512], bf16)

    with tile.TileContext(nc) as tc:
        with tc.tile_pool(name="const", bufs=1) as cpool, \
             tc.tile_pool(name="xblk", bufs=1) as xpool, \
             tc.tile_pool(name="epool", bufs=1) as epool, \
             tc.tile_pool(name="stg", bufs=1) as gpool:

            onesb = cpool.tile([1, 128], bf16)
            nc.vector.memset(onesb[:], 1.0)
            sd_b = cpool.tile([128, 32], bf16)
            nc.scalar.dma_start(sd_b[:], sd_im.ap())
            pbt_sb = cpool.tile([128, 256], bf16)
            nc.scalar.dma_start(pbt_sb[:], pbt.ap())
            pwbb_sb = cpool.tile([64, 1], f32)
            nc.scalar.dma_start(pwbb_sb[:], pwbb.ap())
            pbv = cpool.tile([128, 4], f32)     # epilogue bias per psum bank
            dwT = cpool.tile([64, 72 * 64], bf16)   # [mc, (n,t)*64+cl]
            pkT = cpool.tile([64, 512], bf16)       # [mc, n*64+oc]
            e_ts = [epool.tile([128, 9 * 128], bf16, name=f"e{qs}")
                    for qs in range(2)]

            # x tiles: pair p holds samples (2p, 2p+1), this core's 64ch,
            # split into 4 row-bands so the conv can start on band 0 while
            # the rest of the image is still streaming in.
            xts = []
            for gp in range(4):
                xts.append([xpool.tile([128, XW[b]], bf16,
                                       tag=f"x{b}", bufs=4,
                                       name=f"x{gp}_{b}")
                            for b in range(4)])

            # ================= stage A (scoped pools) ======================
            with tc.tile_pool(name="sa", bufs=1) as apool, \
                 tc.tile_pool(name="wts", bufs=1) as wpool, \
                 tc.tile_pool(name="ps_a", bufs=1, space="PSUM") as ps_a:
                s_sb = apool.tile([128, 16 * 72], bf16)
                nc.sync.dma_start(
                    s_sb[:].rearrange("p (k c) -> p k c", k=16),
                    s_im.ap().rearrange("k p c -> p k c"))

                # ---- pb = pwb_w^T @ s_d + pwb_b  -> pbv [128, 4] ----------
                ps_pb = ps_a.tile([64, 8], f32, tag="psb", bufs=1)
                for kc in range(4):
                    nc.tensor.matmul(
                        ps_pb[:], pbt_sb[:, kc * 64:(kc + 1) * 64],
                        sd_b[:, kc * 8:(kc + 1) * 8],
                        start=(kc == 0), stop=(kc == 3))
                pb_f = apool.tile([64, 8], f32)
                nc.scalar.activation(pb_f[:], ps_pb[:], ACTF.Identity,
                                     bias=pwbb_sb[:], scale=1.0)
                # psA rows = samples (4qs+0 | 4qs+2); psB = (4qs+1 | 4qs+3)
                for qs in range(2):
                    nc.vector.tensor_copy(pbv[0:64, 2 * qs:2 * qs + 2],
                                          pb_f[:, 4 * qs:4 * qs + 2])
                    nc.vector.tensor_copy(pbv[64:128, 2 * qs:2 * qs + 2],
                                          pb_f[:, 4 * qs + 2:4 * qs + 4])

                for nch in range(8):
                    # bias tiles inline; bufs=4 so the dma only waits on a
                    # long-finished block
                    bia1 = wpool.tile([1, 512], bf16, tag="bia", bufs=2,
                                      name=f"dkb{nch}")
                    nc.scalar.dma_start(bia1[:], dkb.ap()[nch:nch + 1, :])
                    bia2 = wpool.tile([1, 512], bf16, tag="bi2", bufs=2,
                                      name=f"pkb{nch}")
                    nc.scalar.dma_start(bia2[:], pkb.ap()[nch:nch + 1, :])
                    # ---- dw slice block: [72=(n,t), 512=(mc_l,cl)] --------
                    ps_dw = ps_a.tile([72, 512], f32, tag="psa", bufs=3,
                                      name=f"psdw{nch}")
                    # half-block wt transfers (8KB/partition runs) with
                    # bufs=3 so ~2 DMAs stay in flight — bigger runs give
                    # much better per-descriptor DMA efficiency
                    for half in range(2):
                        wq = wpool.tile([128, 4096], bf16, tag="wt",
                                        bufs=3, name=f"wt{nch}_{half}")
                        nc.sync.dma_start(wq[:], wt.ap()[nch, half])
                        for k8 in range(8):
                            kc = half * 8 + k8
                            nc.tensor.matmul(
                                ps_dw[:],
                                s_sb[:, kc * 72:(kc + 1) * 72],
                                wq[:, k8 * 512:(k8 + 1) * 512],
                                start=(kc == 0), stop=False)
                    nc.tensor.matmul(ps_dw[:], onesb[0:1, 0:72],
                                     bia1[:], start=False, stop=True)
                    dw_blk = wpool.tile([72, 512], bf16, tag="dwb", bufs=2,
                                        name=f"dwb{nch}")
                    nc.vector.tensor_copy(dw_blk[:], ps_dw[:])
                    # transpose via DRAM bounce: flat DRAM APs have no
                    # partition-order constraint, so the read can iterate
                    # (m, r, c) and lowers to a few 2D descriptors.
                    nc.scalar.dma_start(dwd.ap()[nch], dw_blk[:])
                    nc.scalar.dma_start(
                        dwT[8 * nch:8 * nch + 8, :]
                        .rearrange("m (r c) -> m r c", c=64),
                        dwd.ap()[nch].rearrange("r (m c) -> m r c", m=8))

                    # ---- pk slice block: [8=n, 512=(mc_l,oc)] -------------
                    ps_pk = ps_a.tile([8, 512], f32, tag="psa", bufs=3,
                                      name=f"pspk{nch}")
                    pkt_sb = wpool.tile([128, 2048], bf16, tag="pkw",
                                        bufs=2, name=f"pkt{nch}")
                    nc.scalar.dma_start(pkt_sb[:], pkt.ap()[nch])
                    for kc in range(4):
                        nc.tensor.matmul(
                            ps_pk[:],
                            sd_b[:, kc * 8:(kc + 1) * 8],
                            pkt_sb[:, kc * 512:(kc + 1) * 512],
                            start=(kc == 0), stop=False)
                    nc.tensor.matmul(ps_pk[:], onesb[0:1, 0:8],
                                     bia2[:], start=False, stop=True)
                    pk_blk = wpool.tile([8, 512], bf16, tag="pkb", bufs=2,
                                        name=f"pkb{nch}")
                    nc.vector.tensor_copy(pk_blk[:], ps_pk[:])
                    nc.scalar.dma_start(pkd.ap()[nch], pk_blk[:])
                    nc.scalar.dma_start(
                        pkT[8 * nch:8 * nch + 8, :]
                        .rearrange("m (n o) -> m n o", o=64),
                        pkd.ap()[nch].rearrange("n (m o) -> m n o", m=8))

                # x strictly after the weight stream: a dummy write (from
                # the last dw block) into each x tile makes every x DMA
                # data-dependent on stage A, so the scheduler cannot
                # front-run these transfers into the wt stream.  qs0's
                # bands first — band 0 of pair 0/1 gates the conv start.
                for gp in range(4):
                    for b in range(4):
                        nc.vector.tensor_copy(xts[gp][b][0:1, 0:1],
                                              dw_blk[0:1, 0:1])
                xoff = [0]
                for b in range(4):
                    xoff.append(xoff[-1] + XW[b])
                for b in range(4):
                    for gp in range(2):
                        nc.sync.dma_start(
                            xts[gp][b][:], xin.ap()[gp, :, xoff[b]:xoff[b + 1]])
                for b in range(4):
                    for gp in range(2, 4):
                        nc.sync.dma_start(
                            xts[gp][b][:], xin.ap()[gp, :, xoff[b]:xoff[b + 1]])

            # ============== E^T tiles: [cl, oc] per (sample, tap) ==========
            # e_ts[qs] layout: rows 0:64 = samples 4qs+0 (cols t*128+0:64)
            # and 4qs+2 (cols t*128+64:128); rows 64:128 = 4qs+1, 4qs+3.
            with tc.tile_pool(name="ps_e", bufs=1, space="PSUM") as ps_e:
                for qs in range(2):
                    for ch in range(2):
                        for tg in range(3):
                            psE = ps_e.tile([128, 192], f32, tag="pse",
                                            bufs=2, name=f"pse{qs}{ch}{tg}")
                            for tl in range(3):
                                t = 3 * tg + tl
                                ne = 4 * qs + 2 * ch
                                no = ne + 1
                                nc.tensor.matmul(
                                    psE[0:64, tl * 64:tl * 64 + 64],
                                    dwT[:, (ne * 9 + t) * 64:
                                        (ne * 9 + t) * 64 + 64],
                                    pkT[:, ne * 64:ne * 64 + 64],
                                    start=True, stop=True,
                                    tile_position=(0, 0))
                                nc.tensor.matmul(
                                    psE[64:128, tl * 64:tl * 64 + 64],
                                    dwT[:, (no * 9 + t) * 64:
                                        (no * 9 + t) * 64 + 64],
                                    pkT[:, no * 64:no * 64 + 64],
                                    start=True, stop=True,
                                    tile_position=(0, 64))
                            ev = e_ts[qs][:].rearrange(
                                "p (t x) -> p t x", x=128)
                            nc.vector.tensor_copy(
                                ev[:, 3 * tg:3 * tg + 3,
                                   ch * 64:ch * 64 + 64],
                                psE[:].rearrange("p (t x) -> p t x", x=64))

            # =================== conv (4-way PE tiling) ====================
            cvstack = tc.tile_pool(name="ps_cv", bufs=1, space="PSUM")
            pcv = cvstack.__enter__()
            for qs in range(2):
                et = e_ts[qs]
                nsc = (NCHUNK + SCN - 1) // SCN
                for sc in range(nsc):
                    gcs = list(range(sc * SCN, min((sc + 1) * SCN, NCHUNK)))
                    pas, pbs = [], []
                    for gc in gcs:
                        pas.append(pcv.tile([128, NCH], f32, tag="psA", bufs=4,
                                            name=f"cvA{qs}_{gc}"))
                        pbs.append(pcv.tile([128, NCH], f32, tag="psB", bufs=4,
                                            name=f"cvB{qs}_{gc}"))
                    for t in range(9):
                        i, j = t // 3, t % 3
                        st0, sp = (t == 0), (t == 8)
                        for k, gc in enumerate(gcs):
                            band = gc // 11
                            XA = xts[2 * qs][band]
                            XB = xts[2 * qs + 1][band]
                            r0 = gc * RPC
                            nr = min(RPC, H - r0)
                            N = nr * PW - (2 if r0 + nr >= H else 0)
                            off = (r0 + i - XBASE[band]) * PW + j
                            psA, psB = pas[k], pbs[k]
                            nc.tensor.matmul(
                                psA[0:64, 0:N], et[0:64, t * 128:t * 128 + 64],
                                XA[0:64, off:off + N], start=st0, stop=sp,
                                tile_position=(0, 0))
                            nc.tensor.matmul(
                                psB[0:64, 0:N],
                                et[64:128, t * 128:t * 128 + 64],
                                XA[64:128, off:off + N], start=st0, stop=sp,
                                tile_position=(64, 0))
                            nc.tensor.matmul(
                                psA[64:128, 0:N],
                                et[0:64, t * 128 + 64:t * 128 + 128],
                                XB[0:64, off:off + N], start=st0, stop=sp,
                                tile_position=(0, 64))
                            nc.tensor.matmul(
                                psB[64:128, 0:N],
                                et[64:128, t * 128 + 64:t * 128 + 128],
                                XB[64:128, off:off + N], start=st0, stop=sp,
                                tile_position=(64, 64))
                    # epilogue: strip halo cols, +bias, stage as bf16
                    # (split between ACT and DVE so neither backpressures)
                    stgA = gpool.tile([128, SCN * RPC * 128], bf16, tag="sg",
                                      bufs=3, name=f"stA{qs}_{sc}")
                    stgB = gpool.tile([128, SCN * RPC * 128], bf16, tag="sg",
                                      bufs=3, name=f"stB{qs}_{sc}")
                    cols = 0
                    for k, gc in enumerate(gcs):
                        nr = min(RPC, H - gc * RPC)
                        for ab, (ps, stg) in enumerate(
                                ((pas[k], stgA), (pbs[k], stgB))):
                            q = 2 * qs + ab
                            src = ps[:, 0:nr * PW].rearrange(
                                "p (r c) -> p r c", c=PW)[:, :, 0:128]
                            dst = stg[:, cols:cols + nr * 128].rearrange(
                                "p (r c) -> p r c", c=128)
                            if (k + ab) % 2 == 0:
                                nc.scalar.activation(dst, src, ACTF.Identity,
                                                     bias=pbv[:, q:q + 1],
                                                     scale=1.0)
                            else:
                                nc.vector.tensor_scalar(
                                    dst, src, pbv[:, q:q + 1], None,
                                    op0=ALU.add)
                        cols += nr * 128
                    o0 = sc * SCN * RPC * 128
                    nc.sync.dma_start(out.ap()[2 * qs, :, o0:o0 + cols],
                                      stgA[:, 0:cols])
                    nc.sync.dma_start(out.ap()[2 * qs + 1, :, o0:o0 + cols],
                                      stgB[:, 0:cols])
            cvstack.__exit__(None, None, None)

    nc.compile()
    return nc


def _host_prep(style_encoding, dk_w, dk_b, pwk_w, pwk_b, pwb_w, pwb_b):
    """Per-core weight shards (reshapes/transposes/casts only)."""
    f = np.float32
    bf = ml_dtypes.bfloat16
    st = np.asarray(style_encoding, f)                      # [8, 512, 4, 4]
    WTf = np.asarray(dk_w, f).reshape(32768, KM).T          # [2048, 32768]
    PKTf = np.asarray(pwk_w, f).reshape(32768, 512).T       # [512, 32768]
    pkb_f = np.asarray(pwk_b, f)
    PBT = np.ascontiguousarray(np.asarray(pwb_w, f).reshape(512, 512).T)
    pwb_bf = np.asarray(pwb_b, f)
    dkb_f = np.asarray(dk_b, f)

    # style-tap matrix for the dw GEMM: rows k = ci*4 + khw, cols = n*9 + t
    S = np.empty((KM, 72), f)
    for kh in range(2):
        for kw in range(2):
            blk = st[:, :, kh:kh + 3, kw:kw + 3].reshape(8, 512, 9)
            S[kh * 2 + kw::4, :] = blk.transpose(1, 0, 2).reshape(512, 72)
    S = np.ascontiguousarray(S.reshape(16, 128, 72)).astype(bf)

    # s_d (global mean of the 4x4 style map): [128, kc*8 + n]
    sdvec = st.mean(axis=(2, 3))                            # [8, 512]
    sd_g = np.ascontiguousarray(
        sdvec.T.reshape(4, 128, 8).transpose(1, 0, 2)).reshape(128, 32)
    sd_g = sd_g.astype(bf)

    shards = []
    for g in range(N_CORES):
        sl = slice(g * OCS, (g + 1) * OCS)
        # dw weights: [nch, half, 128, (k8, 512)] — cols (mc_l, cl)
        wtg = np.ascontiguousarray(
            WTf[:, sl].reshape(2, 8, 128, 8, 512).transpose(3, 0, 2, 1, 4)
        ).reshape(8, 2, 128, 4096).astype(bf)
        # pk weights: device cols (mc_l, oc) per block (mc = 8*nch + mc_l)
        PKc = PKTf[:, sl].reshape(512, 64, 64)              # [sd, oc, mc]
        PKp = PKc.transpose(0, 2, 1).reshape(512, 8, 512)   # [sd, b, (m,oc)]
        pktg = np.ascontiguousarray(
            PKp.reshape(4, 128, 8, 512).transpose(2, 1, 0, 3)
        ).reshape(8, 128, 2048).astype(bf)
        pkb_g = np.ascontiguousarray(
            pkb_f[sl].reshape(64, 64).T).reshape(8, 512).astype(bf)
        # pb predictor slice: [128, kc*64 + oc]
        pbt_g = np.ascontiguousarray(
            PBT[:, g * 64:(g + 1) * 64].reshape(4, 128, 64)
            .transpose(1, 0, 2)).reshape(128, 256).astype(bf)
        pwbb_g = np.ascontiguousarray(
            pwb_bf[g * 64:(g + 1) * 64].reshape(64, 1))
        shards.append(dict(
            wt=wtg, pkt=pktg, pbt=pbt_g, s_im=S, sd_im=sd_g,
            dkb=np.ascontiguousarray(dkb_f[sl]).reshape(8, 512).astype(bf),
            pkb=pkb_g, pwbb=pwbb_g,
        ))
    return shards


def _prep_x(predicted, norm):
    """Normalize+pad+cast on host -> per-core [4, 128, XTW] bf16 tiles
    (rows 0..67 then rows 63..129 of the padded image, per sample pair)."""
    f = np.float32
    bf = ml_dtypes.bfloat16
    x = np.asarray(predicted, f).reshape(N_CORES, C, H, W)
    if norm:
        mu = x.mean(axis=(1, 2, 3), keepdims=True)
        sd = np.sqrt(x.var(axis=(1, 2, 3), keepdims=True) + EPS)
        x = (x - mu) / sd
    xp = np.pad(x, ((0, 0), (0, 0), (1, 1), (1, 1)), mode='reflect')
    xp = xp.astype(bf)                                       # [8,512,130,130]
    xins = []
    for g in range(N_CORES):
        # pair p holds samples (2p, 2p+1), channels [64g, 64g+64)
        xg = xp[:, 64 * g:64 * g + 64].reshape(4, 128, (H + 2) * PW)
        xo = np.empty((4, 128, XTW), bf)
        o = 0
        for b in range(4):
            s = XBASE[b] * PW
            xo[:, :, o:o + XW[b]] = xg[:, :, s:s + XW[b]]
            o += XW[b]
        xins.append(xo)
    return xins


def kernel(style_encoding, predicted, dk_w, dk_b, pwk_w, pwk_b, pwb_w, pwb_b,
           norm=True, **_ignored):
    from concourse import bass_utils

    norm = bool(norm)
    if "nc" not in _CACHE:
        _CACHE["nc"] = _build()
    nc = _CACHE["nc"]

    shards = _host_prep(style_encoding, dk_w, dk_b, pwk_w, pwk_b,
                        pwb_w, pwb_b)
    xins = _prep_x(predicted, norm)
    in_maps = []
    for g in range(N_CORES):
        m = dict(shards[g])
        m["xin"] = xins[g]
        in_maps.append(m)

    res = bass_utils.run_bass_kernel_spmd(nc, in_maps,
                                          core_ids=list(range(N_CORES)))
    return _gather(res)


def _gather(res):
    out = np.empty((N_CORES, C, H * W), np.float32)
    for g in range(N_CORES):
        ob = np.asarray(res.results[g]["out"]).astype(np.float32)
        for qs in range(2):
            out[4 * qs + 0, 64 * g:64 * g + 64] = ob[2 * qs, 0:64]
            out[4 * qs + 2, 64 * g:64 * g + 64] = ob[2 * qs, 64:128]
            out[4 * qs + 1, 64 * g:64 * g + 64] = ob[2 * qs + 1, 0:64]
            out[4 * qs + 3, 64 * g:64 * g + 64] = ob[2 * qs + 1, 64:128]
    return out.reshape(N_CORES, C, H, W)
